# revision 42
# baseline (speedup 1.0000x reference)
"""BitLinear (LayerNorm + 8-bit act quant + ternary weight quant + GEMM) on 8 TRN2 cores.

Sharding: data-parallel over flattened rows (B*S = 8192 -> 1024 rows/core).
Each core holds the full quantized transposed weight (bf16, exact for ternary
values) and computes LN + activation quantization for its own rows only, so
there is no redundant vector work and no collective.

Weight quantization (absmean ternary) is precomputed on the host: the weight
scale gamma is a single global scalar and the quantized weights are static --
the standard BitLinear inference setup (the sharding hint explicitly allows
precomputing the weight scale). The weight is also pre-tiled on the host so
every device DMA is a single fully contiguous stream.

Numerics: x_q in [-127,127] and w_q in {-1,0,1} are exact in bf16; products
(<=127) and fp32 PSUM accumulation (sums < 2^19) are exact, so the GEMM is
bit-exact integer arithmetic. Rounding uses the +/-1.5*2^23 trick, which is
round-to-nearest-even like jnp.round.
"""

import numpy as np
import ml_dtypes

import concourse.bass as bass
import concourse.bacc as bacc
import concourse.mybir as mybir
import concourse.tile as tile
from concourse.bass_utils import run_bass_kernel_spmd
from concourse.masks import make_identity

# Problem shapes (hardcoded per contract -- kernel.py must be self-contained).
B, S, K, N = 2, 4096, 2048, 8192
M_TOTAL = B * S              # 8192 flattened rows
N_CORES = 8
M_LOC = M_TOTAL // N_CORES   # 1024 rows per core
P = 128                      # partitions
M_TILES = M_LOC // P         # 8
K_TILES = K // P             # 16
N_MM = 512                   # moving-operand free dim per matmul (1 PSUM bank)
N_CHUNK = 1024               # weight-stream / output-store chunk along N
N_CHUNKS = N // N_CHUNK      # 8

EPS_LN = 1e-5
EPS_Q = 1e-5
MAGIC = 12582912.0           # 1.5 * 2**23: fp32 add/sub performs round-to-nearest-even

FP32 = mybir.dt.float32
BF16 = mybir.dt.bfloat16


def _build_nc(reps=1, dma_transpose=True, bias_on_pool=True, batched_transpose=True,
              psum_bufs=4, split_phases=False, mm_order="kt_outer", ablate=()):
    """ablate: subset of {'transpose','phase_a','store','gemm','epilogue'} --
    drops that piece of work (results wrong) for HW time attribution."""
    nc = bacc.Bacc("TRN2", target_bir_lowering=False, debug=False, num_devices=N_CORES)

    xc_d = nc.dram_tensor("xc", [M_LOC, K], FP32, kind="ExternalInput")
    # host-pretiled weight in 512-wide sub-chunks: [s, p, kt, 512], s = 2*nch + half,
    # so each sub-chunk DMA is one contiguous 2MB stream
    wt_d = nc.dram_tensor(
        "wt", [2 * N_CHUNKS, P, K_TILES, N_MM], BF16, kind="ExternalInput"
    )
    gs_d = nc.dram_tensor("gs", [P, 1], FP32, kind="ExternalInput")   # gamma/127 replicated
    bs_d = nc.dram_tensor("bs", [N], FP32, kind="ExternalInput")
    out_d = nc.dram_tensor("out", [M_LOC, N], FP32, kind="ExternalOutput")

    with tile.TileContext(nc) as tc:
        with (
            tc.tile_pool(name="singles", bufs=1) as singles,
            tc.tile_pool(name="xin", bufs=2) as xin_pool,
            tc.tile_pool(name="xn", bufs=2) as xn_pool,
            tc.tile_pool(name="xq", bufs=2) as xq_pool,
            tc.tile_pool(name="stats", bufs=4) as stats_pool,
            tc.tile_pool(name="xqt", bufs=1) as xqt_pool,
            tc.tile_pool(name="wstream", bufs=5) as w_pool,
            tc.tile_pool(name="osb", bufs=3) as o_pool,
            tc.tile_pool(name="psg", bufs=psum_bufs, space="PSUM") as psg_pool,
        ):
            # --- constants ---
            gs_t = singles.tile([P, 1], FP32)
            nc.sync.dma_start(gs_t[:], gs_d[:])
            bias_t = singles.tile([P, N], FP32)
            xq_all = (
                xqt_pool.tile([P, M_TILES, K], BF16, tag="xqall")
                if split_phases
                else None
            )
            rs_all = singles.tile([P, M_TILES], FP32)   # per-row output scale
            eps_t = singles.tile([P, 1], FP32)
            nc.vector.memset(eps_t[:], EPS_LN)
            if not dma_transpose:
                ident = singles.tile([P, P], BF16)
                make_identity(nc, ident)
            # x_q^T, bf16, laid out [p=k_inner, m_tile, k_tile, m_inner] --
            # each m_tile's transpose write is one contiguous range per
            # partition, so Tile's interval dep-tracking lets m=0 matmuls
            # start while later m tiles are still in phase A
            xqt = xqt_pool.tile([P, M_TILES, K_TILES, P], BF16)
            if "phase_a" in ablate:
                nc.vector.memset(xqt[:], 0.0)
                nc.vector.memset(rs_all[:], 1.0)

            for _rep in range(reps):

                def load_sub(s, hold_ms=None):
                    t = w_pool.tile([P, K_TILES, N_MM], BF16, tag="wstream")
                    if hold_ms is None:
                        nc.sync.dma_start(t[:], wt_d[s])
                    else:
                        # keep this prefetch out of the head window so the
                        # first LN chain isn't starved of SBUF bandwidth
                        with tc.tile_wait_until(hold_ms):
                            nc.sync.dma_start(t[:], wt_d[s])
                    return t

                early_subs = None

                # --- phase A: LayerNorm + activation quant + transpose ---
                for m in range(M_TILES) if "phase_a" not in ablate else ():
                    xt = xin_pool.tile([P, K], FP32)
                    nc.sync.dma_start(xt[:], xc_d[m * P : (m + 1) * P, :])

                    if m == 0 and "gemm" not in ablate:
                        # queue the first two logical chunks' weights (4 x 2MB)
                        # right behind the x0 load: x0 lands first so the LN
                        # chain starts at once; subs 2,3 (needed only at the
                        # second gemm unit, ~55us) held past the LN-critical
                        # first ~12us
                        early_subs = [
                            load_sub(s, hold_ms=None if s < 2 else 0.012)
                            for s in range(4)
                        ]

                    st = stats_pool.tile([P, K // 512, 6], FP32)
                    for i in range(K // 512):
                        nc.vector.bn_stats(st[:, i, :], xt[:, i * 512 : (i + 1) * 512])
                    mv = stats_pool.tile([P, 2], FP32)
                    nc.vector.bn_aggr(mv[:], st[:])

                    # rstd = 1/sqrt(var + eps)  (ACT only ever runs Sqrt-family here)
                    std = stats_pool.tile([P, 1], FP32)
                    nc.scalar.activation(
                        std[:], mv[:, 1:2], mybir.ActivationFunctionType.Sqrt,
                        bias=eps_t[:],
                    )
                    rstd = stats_pool.tile([P, 1], FP32)
                    nc.vector.reciprocal(rstd[:], std[:])

                    # xn = (x - mu) * rstd
                    xn = xn_pool.tile([P, K], FP32)
                    nc.vector.tensor_scalar(
                        out=xn[:],
                        in0=xt[:],
                        scalar1=mv[:, 0:1],
                        scalar2=rstd[:],
                        op0=mybir.AluOpType.subtract,
                        op1=mybir.AluOpType.mult,
                    )

                    # eta = max(absmax(xn), EPS_Q); inv = 127/eta; rs = eta*gamma/127
                    eta = stats_pool.tile([P, 1], FP32)
                    nc.vector.tensor_reduce(
                        out=eta[:], in_=xn[:], axis=mybir.AxisListType.X,
                        op=mybir.AluOpType.max, apply_absolute_value=True,
                    )
                    eta2 = stats_pool.tile([P, 1], FP32)
                    nc.vector.tensor_scalar_max(out=eta2[:], in0=eta[:], scalar1=EPS_Q)
                    inv = stats_pool.tile([P, 1], FP32)
                    nc.vector.reciprocal(inv[:], eta2[:])
                    inv127 = stats_pool.tile([P, 1], FP32)
                    nc.vector.tensor_scalar_mul(out=inv127[:], in0=inv[:], scalar1=127.0)
                    nc.vector.tensor_mul(rs_all[:, m : m + 1], eta2[:], gs_t[:])

                    # round to int (RNE): xn <- xn*inv127 + MAGIC (fp32, exact int+MAGIC)
                    nc.vector.tensor_scalar(
                        out=xn[:], in0=xn[:], scalar1=inv127[:], scalar2=MAGIC,
                        op0=mybir.AluOpType.mult, op1=mybir.AluOpType.add,
                    )
                    # xq_row = xn - MAGIC, downcast bf16 (ints <= 127: exact)
                    xq = xq_all[:, m, :] if split_phases else xq_pool.tile([P, K], BF16)
                    nc.vector.tensor_scalar(
                        out=xq[:], in0=xn[:], scalar1=-MAGIC, scalar2=None,
                        op0=mybir.AluOpType.add,
                    )

                    if split_phases or "transpose" in ablate:
                        pass
                    elif dma_transpose and batched_transpose:
                        # one xbar-transpose for the whole [P, K] tile; probe-
                        # verified: out[p, kt, :] = xq^T[kt*P + p, :], i.e. the
                        # same layout as 16 per-kt square transposes
                        nc.sync.dma_start_transpose(out=xqt[:, m, :, :], in_=xq[:])
                    elif dma_transpose:
                        for kt in range(K_TILES):
                            nc.sync.dma_start_transpose(
                                out=xqt[:, m, kt, :],
                                in_=xq[:, kt * P : (kt + 1) * P],
                            )
                    else:
                        for kt in range(K_TILES):
                            ps = psg_pool.tile([P, P], BF16, tag="pst")
                            nc.tensor.transpose(
                                ps[:], xq[:, kt * P : (kt + 1) * P], ident[:]
                            )
                            nc.vector.tensor_copy(xqt[:, m, kt, :], ps[:])

                if split_phases and "transpose" not in ablate:
                    for m in range(M_TILES) if "phase_a" not in ablate else ():
                        nc.sync.dma_start_transpose(
                            out=xqt[:, m, :, :], in_=xq_all[:, m, :]
                        )

                if _rep == 0:
                    # emitted after phase A so the head DMAs (x tile 0, first
                    # weight half) aren't queued behind this 4MB broadcast
                    bias_bcast = bass.AP(
                        tensor=bs_d.ap().tensor, offset=0, ap=[[0, P]] + bs_d.ap().ap
                    )
                    nc.sync.dma_start(bias_t[:], bias_bcast)

                # --- phase B: GEMM + epilogue ---
                # One (nch, m) unit: 2-bank PSUM tile, kt-outer so consecutive
                # MMs alternate banks (drain/fill overlap), then scale+bias+store.
                def gemm_unit(nch, m, wa, wb):
                    osb = o_pool.tile([P, N_CHUNK], FP32)
                    pt = psg_pool.tile([P, N_CHUNK], FP32)
                    for kt in range(K_TILES):
                        for nj, wsub in ((0, wa), (1, wb)):
                            nc.tensor.matmul(
                                pt[:, nj * N_MM : (nj + 1) * N_MM],
                                xqt[:, m, kt, :],
                                wsub[:, kt, :],
                                start=(kt == 0),
                                stop=(kt == K_TILES - 1),
                            )
                    if "epilogue" in ablate:
                        return
                    # scale by per-row rs (ACT only ever runs Copy here)
                    nc.scalar.activation(
                        osb[:], pt[:],
                        mybir.ActivationFunctionType.Copy,
                        scale=rs_all[:, m : m + 1],
                    )
                    badd = nc.gpsimd if bias_on_pool else nc.vector
                    badd.tensor_add(
                        osb[:],
                        osb[:],
                        bias_t[:, nch * N_CHUNK : (nch + 1) * N_CHUNK],
                    )
                    if "store" in ablate:
                        return
                    nc.sync.dma_start(
                        out_d[m * P : (m + 1) * P,
                              nch * N_CHUNK : (nch + 1) * N_CHUNK],
                        osb[:],
                    )

                if "gemm" not in ablate:
                    # chunks 0+1 interleaved m-wise: 2x the matmul work per
                    # ready m-tile, so the PE rides out phase A's production
                    # rate without starving
                    for m in range(M_TILES):
                        gemm_unit(0, m, early_subs[0], early_subs[1])
                        gemm_unit(1, m, early_subs[2], early_subs[3])
                    # steady phase: chunks 2..7, weight prefetch one ahead
                    for nch in range(2, N_CHUNKS):
                        wa = load_sub(2 * nch, hold_ms=0.035 if nch == 2 else None)
                        wb = load_sub(2 * nch + 1)
                        for m in range(M_TILES):
                            gemm_unit(nch, m, wa, wb)

    nc.compile()
    return nc


_NC_CACHE = None


def _get_nc():
    global _NC_CACHE
    if _NC_CACHE is None:
        _NC_CACHE = _build_nc()
    return _NC_CACHE


def _weight_gamma(weight: np.ndarray) -> np.float32:
    """absmean scale, matching jnp.maximum(jnp.mean(jnp.abs(w)), EPS_Q) bitwise
    where possible (jax-cpu), falling back to float64 numpy."""
    try:
        import jax
        import jax.numpy as jnp

        w_cpu = jax.device_put(np.asarray(weight), jax.devices("cpu")[0])
        g = jnp.maximum(jnp.mean(jnp.abs(w_cpu)), EPS_Q)
        return np.float32(np.asarray(g))
    except Exception:
        return np.float32(max(np.mean(np.abs(weight), dtype=np.float64), EPS_Q))


def _prep_weight(weight: np.ndarray):
    gamma = _weight_gamma(weight)
    w_q = np.round(np.clip(weight.astype(np.float32) / gamma, -1.0, 1.0))
    # [N, K] -> wT [K, N] -> sub-chunk tiles [s, p, kt, 512] with s = 2*nch+half,
    # contiguous per 2MB sub-chunk
    wt = np.ascontiguousarray(w_q.T)                        # [K, N]
    wt = wt.reshape(K_TILES, P, 2 * N_CHUNKS, N_MM)         # [kt, p, s, n]
    wt = np.ascontiguousarray(wt.transpose(2, 1, 0, 3))     # [s, p, kt, n]
    return gamma, wt.astype(ml_dtypes.bfloat16)


def make_in_maps(x: np.ndarray, weight: np.ndarray, bias: np.ndarray):
    gamma, wt_bf16 = _prep_weight(weight)
    gs = np.full((P, 1), gamma / np.float32(127.0), dtype=np.float32)
    bias_f = np.ascontiguousarray(bias.astype(np.float32))
    x_flat = np.ascontiguousarray(x.reshape(M_TOTAL, K).astype(np.float32))
    return [
        {
            "xc": x_flat[c * M_LOC : (c + 1) * M_LOC],
            "wt": wt_bf16,
            "gs": gs,
            "bs": bias_f,
        }
        for c in range(N_CORES)
    ]


def kernel(x: np.ndarray, weight: np.ndarray, bias: np.ndarray) -> np.ndarray:
    assert x.shape == (B, S, K) and weight.shape == (N, K) and bias.shape == (N,)

    in_maps = make_in_maps(x, weight, bias)
    nc = _get_nc()
    res = run_bass_kernel_spmd(nc, in_maps, list(range(N_CORES)))
    out = np.concatenate([res.results[c]["out"] for c in range(N_CORES)], axis=0)
    return out.reshape(B, S, N).astype(np.float32, copy=False)

